# revision 11
# baseline (speedup 1.0000x reference)
"""Trainium2 Bass kernel for nn_DotPred (gnn_message_passing).

score[t, e] = sum_d (x[src] - x[dst]) / sqrt(D)
            = (rowsum(x)[src] - rowsum(x)[dst]) / sqrt(D)

Strategy (8 NeuronCores, SPMD):
- Shard the 1.5M flattened edges across 8 cores; replicate node embeddings.
- Phase 1 (device): rowsum table s[n] = sum_d x[n, d], kept in SBUF as
  S[128, 800] with node n at (partition n & 127, column n >> 7).
- Phase 2 (device): per-edge gather of s[src], s[dst] via one-hot matmuls.
  Host pre-sorts each core's edges by (src_block, dst_block) pair
  (block = 4096 nodes = 128 partitions x 32 columns) into 625 groups padded
  to 128-edge tiles (a core-uniform static schedule). Per 128-edge tile:
    PE poly-mm (k=3):  Q[p, e] = 2p*p_e - p_e^2 - p^2 = -(p - p_e)^2
    DVE/ACT relu:      OHP[p, e] = relu(1 + Q) in {0, 1}  (exact one-hot)
    PE select-mm:      RT[e, c] = sum_p OHP[p, e] * S[p, 32*blk + c]
    GPSIMD:            mask[e, t, c] = (iota_c == c_e)
    DVE:               val[e] = sum_c RT[e, c] * mask     (mult + seg-reduce)
  All arithmetic is exact fp32 (integer polynomials < 2^24, one-hot selects).
- Final: (val_src - val_dst) / sqrt(128) on device; host un-permutes.
"""
import math
from contextlib import ExitStack

import numpy as np

import concourse.bass as bass
import concourse.mybir as mybir
from concourse.bass_utils import run_bass_kernel_spmd

P = 128
D = 128
CB = 32             # columns per block
NBLK = 25           # node blocks (4096 nodes each) covering 100096 nodes
N_NODES = 100000
VPAD = 100352       # 784 * 128 = 196 * 512 (embed DMA batches divide evenly)
NCORES = 8
TPB = 16            # tiles per phase-2 batch (one PSUM bank of RT)
CHT = 4             # tiles per poly/relu chunk (one PSUM bank)
NCH = TPB // CHT    # chunks per batch (4)
INV_SQ = 1.0 / math.sqrt(128.0)

F32 = mybir.dt.float32
ALU = mybir.AluOpType
ACTF = mybir.ActivationFunctionType

# relu chunk assignment: which (side, chunk) relus run on DVE (rest on ACT)
DVE_RELU = {(0, 0), (0, 1), (0, 2)}


def _build_nc(n_tiles, sched):
    assert len(sched) == n_tiles and n_tiles % TPB == 0
    nbatch = n_tiles // TPB
    n_emb = VPAD // 512  # embed batches (4 node-tiles each)
    CW = CHT * P         # chunk width in edges (512)

    nc = bass.Bass()
    embeds = nc.declare_dram_parameter("embeds", [VPAD, D], F32, isOutput=False)
    p3_src = nc.declare_dram_parameter("p3_src", [nbatch, 3, TPB * P], F32, isOutput=False)
    p3_dst = nc.declare_dram_parameter("p3_dst", [nbatch, 3, TPB * P], F32, isOutput=False)
    maskS_in = nc.declare_dram_parameter("maskS_in", [nbatch, P, TPB * CB], F32, isOutput=False)
    maskD_in = nc.declare_dram_parameter("maskD_in", [nbatch, P, TPB * CB], F32, isOutput=False)
    lhsT3_in = nc.declare_dram_parameter("lhsT3", [3, P], F32, isOutput=False)
    y = nc.declare_dram_parameter("y", [P, n_tiles], F32, isOutput=True)

    es = ExitStack()
    with es:
        emb0 = es.enter_context(nc.sbuf_tensor([P, 512], F32))
        emb1 = es.enter_context(nc.sbuf_tensor([P, 512], F32))
        S = es.enter_context(nc.sbuf_tensor([P, NBLK * CB], F32))
        lhsT3 = es.enter_context(nc.sbuf_tensor([3, P], F32))
        pS0 = es.enter_context(nc.sbuf_tensor([3, TPB * P], F32))
        pS1 = es.enter_context(nc.sbuf_tensor([3, TPB * P], F32))
        pD0 = es.enter_context(nc.sbuf_tensor([3, TPB * P], F32))
        pD1 = es.enter_context(nc.sbuf_tensor([3, TPB * P], F32))
        ohpS0 = es.enter_context(nc.sbuf_tensor([P, TPB * P], F32))
        ohpS1 = es.enter_context(nc.sbuf_tensor([P, TPB * P], F32))
        ohpD0 = es.enter_context(nc.sbuf_tensor([P, TPB * P], F32))
        ohpD1 = es.enter_context(nc.sbuf_tensor([P, TPB * P], F32))
        maskS0 = es.enter_context(nc.sbuf_tensor([P, TPB * CB], F32))
        maskS1 = es.enter_context(nc.sbuf_tensor([P, TPB * CB], F32))
        maskD0 = es.enter_context(nc.sbuf_tensor([P, TPB * CB], F32))
        maskD1 = es.enter_context(nc.sbuf_tensor([P, TPB * CB], F32))
        dS = es.enter_context(nc.sbuf_tensor([P, TPB * CB], F32))
        dD = es.enter_context(nc.sbuf_tensor([P, TPB * CB], F32))
        valS = es.enter_context(nc.sbuf_tensor([P, n_tiles], F32))
        valD = es.enter_context(nc.sbuf_tensor([P, n_tiles], F32))
        out_sb = es.enter_context(nc.sbuf_tensor([P, n_tiles], F32))
        qS0 = es.enter_context(nc.psum_tensor([P, CW], F32))
        qS1 = es.enter_context(nc.psum_tensor([P, CW], F32))
        qD0 = es.enter_context(nc.psum_tensor([P, CW], F32))
        qD1 = es.enter_context(nc.psum_tensor([P, CW], F32))
        psA0 = es.enter_context(nc.psum_tensor([P, TPB * CB], F32))
        psA1 = es.enter_context(nc.psum_tensor([P, TPB * CB], F32))
        psB0 = es.enter_context(nc.psum_tensor([P, TPB * CB], F32))
        psB1 = es.enter_context(nc.psum_tensor([P, TPB * CB], F32))
        ph1_loadA = es.enter_context(nc.semaphore())
        ph1_loadB = es.enter_context(nc.semaphore())
        ph1_free = es.enter_context(nc.semaphore())
        pre_load = es.enter_context(nc.semaphore())
        ploadA = es.enter_context(nc.semaphore())
        ploadB = es.enter_context(nc.semaphore())
        mloadA = es.enter_context(nc.semaphore())
        mloadB = es.enter_context(nc.semaphore())
        ydone = es.enter_context(nc.semaphore())
        ps_done = es.enter_context(nc.semaphore())
        pd_done = es.enter_context(nc.semaphore())
        r_sv = es.enter_context(nc.semaphore())  # src relus on DVE
        r_sa = es.enter_context(nc.semaphore())  # src relus on ACT
        r_dv = es.enter_context(nc.semaphore())  # dst relus on DVE
        r_da = es.enter_context(nc.semaphore())  # dst relus on ACT
        seldone = es.enter_context(nc.semaphore())
        dvedone = es.enter_context(nc.semaphore())
        vchain = es.enter_context(nc.semaphore())
        fin = es.enter_context(nc.semaphore())
        block = es.enter_context(nc.Block())

        emb_bufs = [emb0, emb1]
        pS_bufs = [pS0, pS1]
        pD_bufs = [pD0, pD1]
        ohpS_bufs = [ohpS0, ohpS1]
        ohpD_bufs = [ohpD0, ohpD1]
        maskS_bufs = [maskS0, maskS1]
        maskD_bufs = [maskD0, maskD1]
        qS = [qS0, qS1]
        qD = [qD0, qD1]
        psA = [psA0, psA1]
        psB = [psB0, psB1]
        ph1_load = [ph1_loadA, ph1_loadB]
        pload = [ploadA, ploadB]
        mload = [mloadA, mloadB]
        NPRE = 1
        # relu ownership: (side, chunk) -> engine 'v' (DVE) or 'a' (ACT)
        def r_owner(side, c):
            return "v" if (side, c) in DVE_RELU else "a"
        R_SEMS = {(0, "v"): r_sv, (0, "a"): r_sa, (1, "v"): r_dv, (1, "a"): r_da}
        R_PERB = {k: sum(1 for c in range(NCH) if r_owner(k[0], c) == k[1])
                  for k in R_SEMS}
        def r_cum(side, i, c):
            # cumulative count on (side, owner(side, c)) up to and incl (i, c)
            eng = r_owner(side, c)
            n = R_PERB[(side, eng)] * i
            n += sum(1 for cc in range(c + 1) if r_owner(side, cc) == eng)
            return R_SEMS[(side, eng)], n
        def wait_relus_done(eng_obj, side, i):
            # all of batch i's relus for `side` complete
            for e in ("v", "a"):
                pb = R_PERB[(side, e)]
                if pb:
                    eng_obj.wait_ge(R_SEMS[(side, e)], pb * (i + 1))

        @block.sync
        def _(sync):
            sync.dma_start(out=lhsT3[:], in_=lhsT3_in[:]).then_inc(pre_load, 16)
            for k in range(n_emb):
                if k >= 2:
                    sync.wait_ge(ph1_free, k - 1)
                sync.dma_start(
                    out=emb_bufs[k % 2][:],
                    in_=embeds[k * 512:(k + 1) * 512, :].rearrange(
                        "(j p) d -> p j d", p=P
                    ),
                ).then_inc(ph1_load[k % 2], 16)
            for i in range(nbatch):
                if i >= 2:
                    sync.wait_ge(ps_done, NCH * (i - 1))
                    sync.wait_ge(pd_done, NCH * (i - 1))
                sync.dma_start(out=pS_bufs[i % 2][:], in_=p3_src[i]).then_inc(pload[i % 2], 16)
                sync.dma_start(out=pD_bufs[i % 2][:], in_=p3_dst[i]).then_inc(pload[i % 2], 16)
                if i >= 2:
                    sync.wait_ge(dvedone, i - 1)
                sync.dma_start(out=maskS_bufs[i % 2][:], in_=maskS_in[i]).then_inc(mload[i % 2], 16)
                sync.dma_start(out=maskD_bufs[i % 2][:], in_=maskD_in[i]).then_inc(mload[i % 2], 16)
            sync.wait_ge(fin, 1)
            sync.dma_start(out=y[:], in_=out_sb[:]).then_inc(ydone, 16)

        def relu_dve(vector, i, side, c):
            q = (qS if side == 0 else qD)[(i * NCH + c) % 2]
            ohp = (ohpS_bufs if side == 0 else ohpD_bufs)[i % 2]
            sem, val = r_cum(side, i, c)
            vector.tensor_scalar(
                out=ohp[:, c * CW:(c + 1) * CW],
                in0=q[:],
                scalar1=1.0,
                scalar2=0.0,
                op0=ALU.add,
                op1=ALU.max,
            ).then_inc(sem, 1)

        def relu_act(scalar, i, side, c):
            q = (qS if side == 0 else qD)[(i * NCH + c) % 2]
            ohp = (ohpS_bufs if side == 0 else ohpD_bufs)[i % 2]
            sem, val = r_cum(side, i, c)
            scalar.activation(
                out=ohp[:, c * CW:(c + 1) * CW],
                in_=q[:],
                func=ACTF.Relu,
                bias=1.0,
                scale=1.0,
            ).then_inc(sem, 1)

        @block.scalar
        def _(scalar):
            for i in range(nbatch):
                if i >= 2:
                    scalar.wait_ge(seldone, i - 1)  # ohp bufs free
                for c in range(NCH):
                    scalar.wait_ge(pd_done, i * NCH + c + 1)
                    relu_act(scalar, i, 1, c)
                for c in range(NCH):
                    if (0, c) not in DVE_RELU:
                        scalar.wait_ge(ps_done, i * NCH + c + 1)
                        relu_act(scalar, i, 0, c)

        @block.vector
        def _(vector):
            # ---- phase 1 ----
            vector.memset(S[:, 784:NBLK * CB], 0.0)
            for k in range(n_emb):
                vector.wait_ge(ph1_load[k % 2], 16 * (k // 2 + 1))
                vector.tensor_reduce(
                    out=S[:, k * 4:(k + 1) * 4],
                    in_=emb_bufs[k % 2][:].rearrange("p (j d) -> p j d", d=D),
                    op=ALU.add,
                    axis=mybir.AxisListType.X,
                ).then_inc(ph1_free, 1)

            # ---- phase 2 ----
            def relus(i):
                if i >= 2:
                    vector.wait_ge(seldone, i - 1)
                for c in range(NCH):
                    if (0, c) in DVE_RELU:
                        vector.wait_ge(ps_done, i * NCH + c + 1)
                        relu_dve(vector, i, 0, c)

            vch = [0]

            def select(i):
                vector.wait_ge(seldone, i + 1)
                vector.wait_ge(mload[i % 2], 32 * (i // 2 + 1))
                if i >= 1:
                    vector.wait_ge(vchain, vch[0])  # dS WAR vs reduce_s(i-1)
                vector.tensor_tensor(
                    out=dS[:], in0=psA[i % 2][:], in1=maskS_bufs[i % 2][:],
                    op=ALU.mult,
                ).then_inc(vchain, 1)
                vch[0] += 1
                vector.wait_ge(vchain, vch[0])      # dS RAW
                vector.tensor_reduce(
                    out=valS[:, i * TPB:(i + 1) * TPB],
                    in_=dS[:].rearrange("p (t c) -> p t c", c=CB),
                    op=ALU.add,
                    axis=mybir.AxisListType.X,
                ).then_inc(vchain, 1)
                vch[0] += 1
                if i >= 1:
                    vector.wait_ge(dvedone, i)      # dD WAR vs reduce_d(i-1)
                vector.tensor_tensor(
                    out=dD[:], in0=psB[i % 2][:], in1=maskD_bufs[i % 2][:],
                    op=ALU.mult,
                ).then_inc(vchain, 1)
                vch[0] += 1
                vector.wait_ge(vchain, vch[0])      # dD RAW
                vector.tensor_reduce(
                    out=valD[:, i * TPB:(i + 1) * TPB],
                    in_=dD[:].rearrange("p (t c) -> p t c", c=CB),
                    op=ALU.add,
                    axis=mybir.AxisListType.X,
                ).then_inc(dvedone, 1)

            for i in range(nbatch):
                relus(i)
                if i >= 1:
                    select(i - 1)
            select(nbatch - 1)
            vector.wait_ge(vchain, vch[0])
            vector.wait_ge(dvedone, nbatch)
            vector.tensor_tensor(
                out=out_sb[:], in0=valS[:], in1=valD[:], op=ALU.subtract,
            ).then_inc(vchain, 1)
            vch[0] += 1
            vector.wait_ge(vchain, vch[0])
            vector.tensor_scalar(
                out=out_sb[:], in0=out_sb[:], scalar1=INV_SQ, scalar2=None,
                op0=ALU.mult,
            ).then_inc(fin, 1)

        @block.tensor
        def _(tensor):
            tensor.wait_ge(ph1_free, n_emb)
            tensor.wait_ge(pre_load, 16 * NPRE)
            for i in range(nbatch):
                tensor.wait_ge(pload[i % 2], 32 * (i // 2 + 1))
                for c in range(NCH):
                    q = i * NCH + c
                    if q >= 2:
                        i2, c2 = divmod(q - 2, NCH)
                        sem2, n2 = r_cum(0, i2, c2)
                        tensor.wait_ge(sem2, n2)  # qS bank free
                    tensor.matmul(
                        out=qS[q % 2][:],
                        lhsT=lhsT3[:],
                        rhs=pS_bufs[i % 2][:, c * CW:(c + 1) * CW],
                        start=True, stop=True,
                    ).then_inc(ps_done, 1)
                for c in range(NCH):
                    q = i * NCH + c
                    if q >= 2:
                        i2, c2 = divmod(q - 2, NCH)
                        sem2, n2 = r_cum(1, i2, c2)
                        tensor.wait_ge(sem2, n2)
                    tensor.matmul(
                        out=qD[q % 2][:],
                        lhsT=lhsT3[:],
                        rhs=pD_bufs[i % 2][:, c * CW:(c + 1) * CW],
                        start=True, stop=True,
                    ).then_inc(pd_done, 1)
                wait_relus_done(tensor, 0, i)
                wait_relus_done(tensor, 1, i)
                if i >= 2:
                    tensor.wait_ge(dvedone, i - 1)
                for j in range(TPB):
                    bs, bd = sched[i * TPB + j]
                    tensor.matmul(
                        out=psA[i % 2][:, j * CB:(j + 1) * CB],
                        lhsT=ohpS_bufs[i % 2][:, j * P:(j + 1) * P],
                        rhs=S[:, bs * CB:(bs + 1) * CB],
                        start=True, stop=True,
                    )
                    mm = tensor.matmul(
                        out=psB[i % 2][:, j * CB:(j + 1) * CB],
                        lhsT=ohpD_bufs[i % 2][:, j * P:(j + 1) * P],
                        rhs=S[:, bd * CB:(bd + 1) * CB],
                        start=True, stop=True,
                    )
                    if j == TPB - 1:
                        mm.then_inc(seldone, 1)

    return nc


def _prep(src_flat, dst_flat):
    E = src_flat.shape[0]
    assert E % NCORES == 0
    Ec = E // NCORES
    NG = NBLK * NBLK

    cores = []
    counts = np.zeros((NCORES, NG), np.int64)
    for i in range(NCORES):
        s = src_flat[i * Ec:(i + 1) * Ec].astype(np.int64)
        d = dst_flat[i * Ec:(i + 1) * Ec].astype(np.int64)
        g = (s >> 12) * NBLK + (d >> 12)
        order = np.argsort(g, kind="stable")
        cores.append((s[order], d[order], g[order], order + i * Ec))
        counts[i] = np.bincount(g, minlength=NG)

    gmax = counts.max(axis=0)
    tiles_per_group = (gmax + P - 1) // P
    n_tiles = int(tiles_per_group.sum())
    n_tiles_p = ((n_tiles + TPB - 1) // TPB) * TPB

    sched = []
    for gi in range(NG):
        sched.extend([(gi // NBLK, gi % NBLK)] * int(tiles_per_group[gi]))
    sched.extend([(0, 0)] * (n_tiles_p - n_tiles))

    slot_base = np.zeros(NG, np.int64)
    np.cumsum(tiles_per_group[:-1] * P, out=slot_base[1:])
    n_slots = n_tiles_p * P
    nbatch = n_tiles_p // TPB

    per_core = []
    for i in range(NCORES):
        s, d, g, orig = cores[i]
        cstart = np.zeros(NG, np.int64)
        np.cumsum(counts[i][:-1], out=cstart[1:])
        within = np.arange(Ec) - cstart[g]
        slot = slot_base[g] + within
        src_s = np.zeros(n_slots, np.int64)
        dst_s = np.zeros(n_slots, np.int64)
        src_s[slot] = s
        dst_s[slot] = d

        def p3(arr):
            pe = (arr & 127).astype(np.float32).reshape(nbatch, TPB * P)
            out = np.empty((nbatch, 3, TPB * P), np.float32)
            out[:, 0, :] = pe
            out[:, 1, :] = -(pe * pe)
            out[:, 2, :] = 1.0
            return out

        def cmask(arr):
            # [nbatch, P(edge-in-tile), TPB*CB]: one-hot of c_e along CB
            c = ((arr >> 7) & 31).astype(np.int8).reshape(nbatch, TPB, P)
            oh = (c[:, :, :, None] == np.arange(CB, dtype=np.int8)).astype(
                np.float32
            )  # [nbatch, TPB, P(e), CB]
            return np.ascontiguousarray(
                oh.transpose(0, 2, 1, 3).reshape(nbatch, P, TPB * CB)
            )

        per_core.append(
            dict(
                p3_src=p3(src_s),
                p3_dst=p3(dst_s),
                maskS=cmask(src_s),
                maskD=cmask(dst_s),
                slot=slot,
                orig=orig,
            )
        )
    return per_core, sched, n_tiles_p


def kernel(node_embeds, src_idx, dst_idx):
    node_embeds = np.asarray(node_embeds, dtype=np.float32)
    src_idx = np.asarray(src_idx)
    dst_idx = np.asarray(dst_idx)
    T, E = src_idx.shape
    n_nodes = node_embeds.shape[0]

    src_flat = src_idx.reshape(-1).astype(np.int64)
    dst_flat = dst_idx.reshape(-1).astype(np.int64)
    per_core, sched, n_tiles_p = _prep(src_flat, dst_flat)

    emb_pad = np.zeros((VPAD, D), np.float32)
    emb_pad[:n_nodes] = node_embeds

    iota = np.arange(P, dtype=np.float32)
    lhsT3 = np.stack([2.0 * iota, np.ones(P, np.float32), -(iota * iota)])

    nc = _build_nc(n_tiles_p, sched)
    in_maps = []
    for i in range(NCORES):
        pc = per_core[i]
        in_maps.append(
            {
                "embeds": emb_pad,
                "p3_src": pc["p3_src"],
                "p3_dst": pc["p3_dst"],
                "maskS_in": pc["maskS"],
                "maskD_in": pc["maskD"],
                "lhsT3": lhsT3,
            }
        )
    res = run_bass_kernel_spmd(nc, in_maps, list(range(NCORES)))

    out_flat = np.zeros(T * E, np.float32)
    for i in range(NCORES):
        pc = per_core[i]
        yv = res.results[i]["y"]
        slot_vals = np.ascontiguousarray(yv.T).reshape(-1)
        out_flat[pc["orig"]] = slot_vals[pc["slot"]]
    return out_flat.reshape(T, E)


# revision 13
# speedup vs baseline: 1.1263x; 1.1263x over previous
"""Trainium2 Bass kernel for nn_DotPred (gnn_message_passing).

score[t, e] = sum_d (x[src] - x[dst]) / sqrt(D)
            = (rowsum(x)[src] - rowsum(x)[dst]) / sqrt(D)

Strategy (8 NeuronCores, SPMD):
- Shard the 1.5M flattened edges across 8 cores; replicate node embeddings.
- Phase 1 (device): rowsum table s[n] = sum_d x[n, d], kept in SBUF as
  S[128, 800] with node n at (partition n & 127, column n >> 7).
- Phase 2 (device): per-edge gather of s[src], s[dst] via one-hot matmuls.
  Host pre-sorts each core's edges by (src_block, dst_block) pair
  (block = 4096 nodes = 128 partitions x 32 columns) into 625 groups padded
  to 128-edge tiles (a core-uniform static schedule). Per 128-edge tile:
    PE poly-mm (k=3):  Q[p, e] = 2p*p_e - p_e^2 - p^2 = -(p - p_e)^2
    DVE/ACT relu:      OHP[p, e] = relu(1 + Q) in {0, 1}  (exact one-hot)
    PE select-mm:      RT[e, c] = sum_p OHP[p, e] * S[p, 32*blk + c]
    GPSIMD:            mask[e, t, c] = (iota_c == c_e)
    DVE:               val[e] = sum_c RT[e, c] * mask     (mult + seg-reduce)
  All arithmetic is exact fp32 (integer polynomials < 2^24, one-hot selects).
- Final: (val_src - val_dst) / sqrt(128) on device; host un-permutes.
"""
import math
from contextlib import ExitStack

import numpy as np

import concourse.bass as bass
import concourse.mybir as mybir
from concourse.bass_utils import run_bass_kernel_spmd

P = 128
D = 128
CB = 32             # columns per block
NBLK = 25           # node blocks (4096 nodes each) covering 100096 nodes
N_NODES = 100000
VPAD = 100352       # 784 * 128 = 196 * 512 (embed DMA batches divide evenly)
NCORES = 8
TPB = 16            # tiles per phase-2 batch (one PSUM bank of RT)
CHT = 4             # tiles per poly/relu chunk (one PSUM bank)
NCH = TPB // CHT    # chunks per batch (4)
INV_SQ = 1.0 / math.sqrt(128.0)

F32 = mybir.dt.float32
ALU = mybir.AluOpType
ACTF = mybir.ActivationFunctionType

# relu chunk assignment: which (side, chunk) relus run on DVE (rest on ACT)
DVE_RELU = {(0, 0), (0, 1), (0, 2)}


def _build_nc(n_tiles, sched):
    assert len(sched) == n_tiles and n_tiles % TPB == 0
    nbatch = n_tiles // TPB
    n_emb = VPAD // 2048  # embed batches (16 node-tiles each)
    CW = CHT * P         # chunk width in edges (512)

    nc = bass.Bass()
    embeds = nc.declare_dram_parameter("embeds", [VPAD, D], F32, isOutput=False)
    p3_both = nc.declare_dram_parameter("p3_both", [nbatch, 6, TPB * P], F32, isOutput=False)
    mask_in = nc.declare_dram_parameter("mask_in", [nbatch, P, 2 * TPB * CB], F32, isOutput=False)
    lhsT3_in = nc.declare_dram_parameter("lhsT3", [3, P], F32, isOutput=False)
    y = nc.declare_dram_parameter("y", [P, n_tiles], F32, isOutput=True)

    es = ExitStack()
    with es:
        emb0 = es.enter_context(nc.sbuf_tensor([P, 2048], F32))
        emb1 = es.enter_context(nc.sbuf_tensor([P, 2048], F32))
        S = es.enter_context(nc.sbuf_tensor([P, NBLK * CB], F32))
        lhsT3 = es.enter_context(nc.sbuf_tensor([3, P], F32))
        pS0 = es.enter_context(nc.sbuf_tensor([3, TPB * P], F32))
        pS1 = es.enter_context(nc.sbuf_tensor([3, TPB * P], F32))
        pD0 = es.enter_context(nc.sbuf_tensor([3, TPB * P], F32))
        pD1 = es.enter_context(nc.sbuf_tensor([3, TPB * P], F32))
        ohpS0 = es.enter_context(nc.sbuf_tensor([P, TPB * P], F32))
        ohpS1 = es.enter_context(nc.sbuf_tensor([P, TPB * P], F32))
        ohpD0 = es.enter_context(nc.sbuf_tensor([P, TPB * P], F32))
        ohpD1 = es.enter_context(nc.sbuf_tensor([P, TPB * P], F32))
        mb0 = es.enter_context(nc.sbuf_tensor([P, 2 * TPB * CB], F32))
        mb1 = es.enter_context(nc.sbuf_tensor([P, 2 * TPB * CB], F32))
        dS = es.enter_context(nc.sbuf_tensor([P, TPB * CB], F32))
        dD = es.enter_context(nc.sbuf_tensor([P, TPB * CB], F32))
        valS = es.enter_context(nc.sbuf_tensor([P, n_tiles], F32))
        valD = es.enter_context(nc.sbuf_tensor([P, n_tiles], F32))
        out_sb = es.enter_context(nc.sbuf_tensor([P, n_tiles], F32))
        qS0 = es.enter_context(nc.psum_tensor([P, CW], F32))
        qS1 = es.enter_context(nc.psum_tensor([P, CW], F32))
        qD0 = es.enter_context(nc.psum_tensor([P, CW], F32))
        qD1 = es.enter_context(nc.psum_tensor([P, CW], F32))
        psA0 = es.enter_context(nc.psum_tensor([P, TPB * CB], F32))
        psA1 = es.enter_context(nc.psum_tensor([P, TPB * CB], F32))
        psB0 = es.enter_context(nc.psum_tensor([P, TPB * CB], F32))
        psB1 = es.enter_context(nc.psum_tensor([P, TPB * CB], F32))
        ph1_loadA = es.enter_context(nc.semaphore())
        ph1_loadB = es.enter_context(nc.semaphore())
        ph1_free = es.enter_context(nc.semaphore())
        pre_load = es.enter_context(nc.semaphore())
        ploadA = es.enter_context(nc.semaphore())
        ploadB = es.enter_context(nc.semaphore())
        mloadA = es.enter_context(nc.semaphore())
        mloadB = es.enter_context(nc.semaphore())
        ydone = es.enter_context(nc.semaphore())
        ps_done = es.enter_context(nc.semaphore())
        pd_done = es.enter_context(nc.semaphore())
        r_sv = es.enter_context(nc.semaphore())  # src relus on DVE
        r_sa = es.enter_context(nc.semaphore())  # src relus on ACT
        r_dv = es.enter_context(nc.semaphore())  # dst relus on DVE
        r_da = es.enter_context(nc.semaphore())  # dst relus on ACT
        seldone = es.enter_context(nc.semaphore())
        dvedone = es.enter_context(nc.semaphore())
        vchain = es.enter_context(nc.semaphore())
        fin = es.enter_context(nc.semaphore())
        block = es.enter_context(nc.Block())

        emb_bufs = [emb0, emb1]
        pS_bufs = [pS0, pS1]
        pD_bufs = [pD0, pD1]
        ohpS_bufs = [ohpS0, ohpS1]
        ohpD_bufs = [ohpD0, ohpD1]
        mb_bufs = [mb0, mb1]
        qS = [qS0, qS1]
        qD = [qD0, qD1]
        psA = [psA0, psA1]
        psB = [psB0, psB1]
        ph1_load = [ph1_loadA, ph1_loadB]
        pload = [ploadA, ploadB]
        mload = [mloadA, mloadB]
        NPRE = 1
        # relu ownership: (side, chunk) -> engine 'v' (DVE) or 'a' (ACT)
        def r_owner(side, c):
            return "v" if (side, c) in DVE_RELU else "a"
        R_SEMS = {(0, "v"): r_sv, (0, "a"): r_sa, (1, "v"): r_dv, (1, "a"): r_da}
        R_PERB = {k: sum(1 for c in range(NCH) if r_owner(k[0], c) == k[1])
                  for k in R_SEMS}
        def r_cum(side, i, c):
            # cumulative count on (side, owner(side, c)) up to and incl (i, c)
            eng = r_owner(side, c)
            n = R_PERB[(side, eng)] * i
            n += sum(1 for cc in range(c + 1) if r_owner(side, cc) == eng)
            return R_SEMS[(side, eng)], n
        def wait_relus_done(eng_obj, side, i):
            # all of batch i's relus for `side` complete
            for e in ("v", "a"):
                pb = R_PERB[(side, e)]
                if pb:
                    eng_obj.wait_ge(R_SEMS[(side, e)], pb * (i + 1))

        @block.sync
        def _(sync):
            sync.dma_start(out=lhsT3[:], in_=lhsT3_in[:]).then_inc(pre_load, 16)
            for k in range(n_emb):
                if k >= 2:
                    sync.wait_ge(ph1_free, k - 1)
                sync.dma_start(
                    out=emb_bufs[k % 2][:],
                    in_=embeds[k * 2048:(k + 1) * 2048, :].rearrange(
                        "(j p) d -> p j d", p=P
                    ),
                ).then_inc(ph1_load[k % 2], 16)
            for i in range(nbatch):
                if i >= 2:
                    sync.wait_ge(ps_done, NCH * (i - 1))
                    sync.wait_ge(pd_done, NCH * (i - 1))

                sync.dma_start(out=pS_bufs[i % 2][:], in_=p3_both[i, 0:3]).then_inc(pload[i % 2], 16)
                sync.dma_start(out=pD_bufs[i % 2][:], in_=p3_both[i, 3:6]).then_inc(pload[i % 2], 16)
                if i >= 2:
                    sync.wait_ge(dvedone, i - 1)
                sync.dma_start(out=mb_bufs[i % 2][:], in_=mask_in[i]).then_inc(mload[i % 2], 16)
            sync.wait_ge(fin, 1)
            sync.dma_start(out=y[:], in_=out_sb[:]).then_inc(ydone, 16)

        def relu_dve(vector, i, side, c):
            q = (qS if side == 0 else qD)[(i * NCH + c) % 2]
            ohp = (ohpS_bufs if side == 0 else ohpD_bufs)[i % 2]
            sem, val = r_cum(side, i, c)
            vector.tensor_scalar(
                out=ohp[:, c * CW:(c + 1) * CW],
                in0=q[:],
                scalar1=1.0,
                scalar2=0.0,
                op0=ALU.add,
                op1=ALU.max,
            ).then_inc(sem, 1)

        def relu_act(scalar, i, side, c):
            q = (qS if side == 0 else qD)[(i * NCH + c) % 2]
            ohp = (ohpS_bufs if side == 0 else ohpD_bufs)[i % 2]
            sem, val = r_cum(side, i, c)
            scalar.activation(
                out=ohp[:, c * CW:(c + 1) * CW],
                in_=q[:],
                func=ACTF.Relu,
                bias=1.0,
                scale=1.0,
            ).then_inc(sem, 1)

        @block.scalar
        def _(scalar):
            for i in range(nbatch):
                if i >= 2:
                    scalar.wait_ge(seldone, i - 1)  # ohp bufs free
                for c in range(NCH):
                    scalar.wait_ge(pd_done, i * NCH + c + 1)
                    relu_act(scalar, i, 1, c)
                for c in range(NCH):
                    if (0, c) not in DVE_RELU:
                        scalar.wait_ge(ps_done, i * NCH + c + 1)
                        relu_act(scalar, i, 0, c)

        @block.vector
        def _(vector):
            # ---- phase 1 ----
            vector.memset(S[:, 784:NBLK * CB], 0.0)
            for k in range(n_emb):
                vector.wait_ge(ph1_load[k % 2], 16 * (k // 2 + 1))
                vector.tensor_reduce(
                    out=S[:, k * 16:(k + 1) * 16],
                    in_=emb_bufs[k % 2][:].rearrange("p (j d) -> p j d", d=D),
                    op=ALU.add,
                    axis=mybir.AxisListType.X,
                ).then_inc(ph1_free, 1)

            # ---- phase 2 ----
            def relus(i):
                if i >= 2:
                    vector.wait_ge(seldone, i - 1)
                for c in range(NCH):
                    if (0, c) in DVE_RELU:
                        vector.wait_ge(ps_done, i * NCH + c + 1)
                        relu_dve(vector, i, 0, c)

            vch = [0]

            def select(i):
                vector.wait_ge(seldone, i + 1)
                vector.wait_ge(mload[i % 2], 16 * (i // 2 + 1))
                if i >= 1:
                    vector.wait_ge(vchain, vch[0])  # dS WAR vs reduce_s(i-1)
                vector.tensor_tensor(
                    out=dS[:], in0=psA[i % 2][:], in1=mb_bufs[i % 2][:, :TPB * CB],
                    op=ALU.mult,
                ).then_inc(vchain, 1)
                vch[0] += 1
                vector.wait_ge(vchain, vch[0])      # dS RAW
                vector.tensor_reduce(
                    out=valS[:, i * TPB:(i + 1) * TPB],
                    in_=dS[:].rearrange("p (t c) -> p t c", c=CB),
                    op=ALU.add,
                    axis=mybir.AxisListType.X,
                ).then_inc(vchain, 1)
                vch[0] += 1
                if i >= 1:
                    vector.wait_ge(dvedone, i)      # dD WAR vs reduce_d(i-1)
                vector.tensor_tensor(
                    out=dD[:], in0=psB[i % 2][:], in1=mb_bufs[i % 2][:, TPB * CB:],
                    op=ALU.mult,
                ).then_inc(vchain, 1)
                vch[0] += 1
                vector.wait_ge(vchain, vch[0])      # dD RAW
                vector.tensor_reduce(
                    out=valD[:, i * TPB:(i + 1) * TPB],
                    in_=dD[:].rearrange("p (t c) -> p t c", c=CB),
                    op=ALU.add,
                    axis=mybir.AxisListType.X,
                ).then_inc(dvedone, 1)

            for i in range(nbatch):
                relus(i)
                if i >= 1:
                    select(i - 1)
            select(nbatch - 1)
            vector.wait_ge(vchain, vch[0])
            vector.wait_ge(dvedone, nbatch)
            vector.tensor_tensor(
                out=out_sb[:], in0=valS[:], in1=valD[:], op=ALU.subtract,
            ).then_inc(vchain, 1)
            vch[0] += 1
            vector.wait_ge(vchain, vch[0])
            vector.tensor_scalar(
                out=out_sb[:], in0=out_sb[:], scalar1=INV_SQ, scalar2=None,
                op0=ALU.mult,
            ).then_inc(fin, 1)

        @block.tensor
        def _(tensor):
            tensor.wait_ge(ph1_free, n_emb)
            tensor.wait_ge(pre_load, 16 * NPRE)
            for i in range(nbatch):
                tensor.wait_ge(pload[i % 2], 32 * (i // 2 + 1))
                for c in range(NCH):
                    q = i * NCH + c
                    if q >= 2:
                        i2, c2 = divmod(q - 2, NCH)
                        sem2, n2 = r_cum(0, i2, c2)
                        tensor.wait_ge(sem2, n2)  # qS bank free
                    tensor.matmul(
                        out=qS[q % 2][:],
                        lhsT=lhsT3[:],
                        rhs=pS_bufs[i % 2][:, c * CW:(c + 1) * CW],
                        start=True, stop=True,
                    ).then_inc(ps_done, 1)
                for c in range(NCH):
                    q = i * NCH + c
                    if q >= 2:
                        i2, c2 = divmod(q - 2, NCH)
                        sem2, n2 = r_cum(1, i2, c2)
                        tensor.wait_ge(sem2, n2)
                    tensor.matmul(
                        out=qD[q % 2][:],
                        lhsT=lhsT3[:],
                        rhs=pD_bufs[i % 2][:, c * CW:(c + 1) * CW],
                        start=True, stop=True,
                    ).then_inc(pd_done, 1)
                wait_relus_done(tensor, 0, i)
                wait_relus_done(tensor, 1, i)
                if i >= 2:
                    tensor.wait_ge(dvedone, i - 1)
                for j in range(TPB):
                    bs, bd = sched[i * TPB + j]
                    tensor.matmul(
                        out=psA[i % 2][:, j * CB:(j + 1) * CB],
                        lhsT=ohpS_bufs[i % 2][:, j * P:(j + 1) * P],
                        rhs=S[:, bs * CB:(bs + 1) * CB],
                        start=True, stop=True,
                    )
                    mm = tensor.matmul(
                        out=psB[i % 2][:, j * CB:(j + 1) * CB],
                        lhsT=ohpD_bufs[i % 2][:, j * P:(j + 1) * P],
                        rhs=S[:, bd * CB:(bd + 1) * CB],
                        start=True, stop=True,
                    )
                    if j == TPB - 1:
                        mm.then_inc(seldone, 1)

    return nc


def _prep(src_flat, dst_flat):
    E = src_flat.shape[0]
    assert E % NCORES == 0
    Ec = E // NCORES
    NG = NBLK * NBLK

    cores = []
    counts = np.zeros((NCORES, NG), np.int64)
    for i in range(NCORES):
        s = src_flat[i * Ec:(i + 1) * Ec].astype(np.int64)
        d = dst_flat[i * Ec:(i + 1) * Ec].astype(np.int64)
        g = (s >> 12) * NBLK + (d >> 12)
        order = np.argsort(g, kind="stable")
        cores.append((s[order], d[order], g[order], order + i * Ec))
        counts[i] = np.bincount(g, minlength=NG)

    gmax = counts.max(axis=0)
    tiles_per_group = (gmax + P - 1) // P
    n_tiles = int(tiles_per_group.sum())
    n_tiles_p = ((n_tiles + TPB - 1) // TPB) * TPB

    sched = []
    for gi in range(NG):
        sched.extend([(gi // NBLK, gi % NBLK)] * int(tiles_per_group[gi]))
    sched.extend([(0, 0)] * (n_tiles_p - n_tiles))

    slot_base = np.zeros(NG, np.int64)
    np.cumsum(tiles_per_group[:-1] * P, out=slot_base[1:])
    n_slots = n_tiles_p * P
    nbatch = n_tiles_p // TPB

    per_core = []
    for i in range(NCORES):
        s, d, g, orig = cores[i]
        cstart = np.zeros(NG, np.int64)
        np.cumsum(counts[i][:-1], out=cstart[1:])
        within = np.arange(Ec) - cstart[g]
        slot = slot_base[g] + within
        src_s = np.zeros(n_slots, np.int64)
        dst_s = np.zeros(n_slots, np.int64)
        src_s[slot] = s
        dst_s[slot] = d

        def p3(arr):
            pe = (arr & 127).astype(np.float32).reshape(nbatch, TPB * P)
            out = np.empty((nbatch, 3, TPB * P), np.float32)
            out[:, 0, :] = pe
            out[:, 1, :] = -(pe * pe)
            out[:, 2, :] = 1.0
            return out

        def cmask(arr):
            # [nbatch, P(edge-in-tile), TPB*CB]: one-hot of c_e along CB
            c = ((arr >> 7) & 31).astype(np.int8).reshape(nbatch, TPB, P)
            oh = (c[:, :, :, None] == np.arange(CB, dtype=np.int8)).astype(
                np.float32
            )  # [nbatch, TPB, P(e), CB]
            return np.ascontiguousarray(
                oh.transpose(0, 2, 1, 3).reshape(nbatch, P, TPB * CB)
            )

        per_core.append(
            dict(
                p3_both=np.concatenate([p3(src_s), p3(dst_s)], axis=1),
                mask_both=np.concatenate([cmask(src_s), cmask(dst_s)], axis=2),
                slot=slot,
                orig=orig,
            )
        )
    return per_core, sched, n_tiles_p


def kernel(node_embeds, src_idx, dst_idx):
    node_embeds = np.asarray(node_embeds, dtype=np.float32)
    src_idx = np.asarray(src_idx)
    dst_idx = np.asarray(dst_idx)
    T, E = src_idx.shape
    n_nodes = node_embeds.shape[0]

    src_flat = src_idx.reshape(-1).astype(np.int64)
    dst_flat = dst_idx.reshape(-1).astype(np.int64)
    per_core, sched, n_tiles_p = _prep(src_flat, dst_flat)

    emb_pad = np.zeros((VPAD, D), np.float32)
    emb_pad[:n_nodes] = node_embeds

    iota = np.arange(P, dtype=np.float32)
    lhsT3 = np.stack([2.0 * iota, np.ones(P, np.float32), -(iota * iota)])

    nc = _build_nc(n_tiles_p, sched)
    in_maps = []
    for i in range(NCORES):
        pc = per_core[i]
        in_maps.append(
            {
                "embeds": emb_pad,
                "p3_both": pc["p3_both"],
                "mask_in": pc["mask_both"],
                "lhsT3": lhsT3,
            }
        )
    res = run_bass_kernel_spmd(nc, in_maps, list(range(NCORES)))

    out_flat = np.zeros(T * E, np.float32)
    for i in range(NCORES):
        pc = per_core[i]
        yv = res.results[i]["y"]
        slot_vals = np.ascontiguousarray(yv.T).reshape(-1)
        out_flat[pc["orig"]] = slot_vals[pc["slot"]]
    return out_flat.reshape(T, E)


# revision 14
# speedup vs baseline: 1.1949x; 1.0609x over previous
"""Trainium2 Bass kernel for nn_DotPred (gnn_message_passing).

score[t, e] = sum_d (x[src] - x[dst]) / sqrt(D)
            = (rowsum(x)[src] - rowsum(x)[dst]) / sqrt(D)

Strategy (8 NeuronCores, SPMD):
- Shard the 1.5M flattened edges across 8 cores; replicate node embeddings.
- Phase 1 (device): rowsum table s[n] = sum_d x[n, d], kept in SBUF as
  S[128, 800] with node n at (partition n & 127, column n >> 7).
- Phase 2 (device): per-edge gather of s[src], s[dst] via one-hot matmuls.
  Host pre-sorts each core's edges by (src_block, dst_block) pair
  (block = 4096 nodes = 128 partitions x 32 columns) into 625 groups padded
  to 128-edge tiles (a core-uniform static schedule). Per 128-edge tile:
    PE poly-mm (k=3):  Q[p, e] = 2p*p_e - p_e^2 - p^2 = -(p - p_e)^2
    DVE/ACT relu:      OHP[p, e] = relu(1 + Q) in {0, 1}  (exact one-hot)
    PE select-mm:      RT[e, c] = sum_p OHP[p, e] * S[p, 32*blk + c]
    GPSIMD:            mask[e, t, c] = (iota_c == c_e)
    DVE:               val[e] = sum_c RT[e, c] * mask     (mult + seg-reduce)
  All arithmetic is exact fp32 (integer polynomials < 2^24, one-hot selects).
- Final: (val_src - val_dst) / sqrt(128) on device; host un-permutes.
"""
import math
from contextlib import ExitStack

import numpy as np

import concourse.bass as bass
import concourse.mybir as mybir
from concourse.bass_utils import run_bass_kernel_spmd

P = 128
D = 128
CB = 32             # columns per block
NBLK = 25           # node blocks (4096 nodes each) covering 100096 nodes
N_NODES = 100000
VPAD = 100352       # 784 * 128 = 196 * 512 (embed DMA batches divide evenly)
NCORES = 8
TPB = 16            # tiles per phase-2 batch (one PSUM bank of RT)
CHT = 4             # tiles per poly/relu chunk (one PSUM bank)
NCH = TPB // CHT    # chunks per batch (4)
INV_SQ = 1.0 / math.sqrt(128.0)

F32 = mybir.dt.float32
ALU = mybir.AluOpType
ACTF = mybir.ActivationFunctionType

# relu chunk assignment: which (side, chunk) relus run on DVE (rest on ACT)
DVE_RELU = {(0, 0), (0, 1), (0, 2), (0, 3)}


def _build_nc(n_tiles, sched):
    assert len(sched) == n_tiles and n_tiles % TPB == 0
    nbatch = n_tiles // TPB
    n_emb = VPAD // 2048  # embed batches (16 node-tiles each)
    CW = CHT * P         # chunk width in edges (512)

    nc = bass.Bass()
    embeds = nc.declare_dram_parameter("embeds", [VPAD, D], F32, isOutput=False)
    p3_both = nc.declare_dram_parameter("p3_both", [nbatch, 6, TPB * P], F32, isOutput=False)
    mask_in = nc.declare_dram_parameter("mask_in", [nbatch, P, 2 * TPB * CB], F32, isOutput=False)
    lhsT3_in = nc.declare_dram_parameter("lhsT3", [3, P], F32, isOutput=False)
    y = nc.declare_dram_parameter("y", [P, n_tiles], F32, isOutput=True)

    es = ExitStack()
    with es:
        emb0 = es.enter_context(nc.sbuf_tensor([P, 2048], F32))
        emb1 = es.enter_context(nc.sbuf_tensor([P, 2048], F32))
        S = es.enter_context(nc.sbuf_tensor([P, NBLK * CB], F32))
        lhsT3 = es.enter_context(nc.sbuf_tensor([3, P], F32))
        pS0 = es.enter_context(nc.sbuf_tensor([3, TPB * P], F32))
        pS1 = es.enter_context(nc.sbuf_tensor([3, TPB * P], F32))
        pD0 = es.enter_context(nc.sbuf_tensor([3, TPB * P], F32))
        pD1 = es.enter_context(nc.sbuf_tensor([3, TPB * P], F32))
        ohpS0 = es.enter_context(nc.sbuf_tensor([P, TPB * P], F32))
        ohpS1 = es.enter_context(nc.sbuf_tensor([P, TPB * P], F32))
        ohpD0 = es.enter_context(nc.sbuf_tensor([P, TPB * P], F32))
        ohpD1 = es.enter_context(nc.sbuf_tensor([P, TPB * P], F32))
        mb0 = es.enter_context(nc.sbuf_tensor([P, 2 * TPB * CB], F32))
        mb1 = es.enter_context(nc.sbuf_tensor([P, 2 * TPB * CB], F32))
        dS = es.enter_context(nc.sbuf_tensor([P, TPB * CB], F32))
        dD = es.enter_context(nc.sbuf_tensor([P, TPB * CB], F32))
        valS = es.enter_context(nc.sbuf_tensor([P, n_tiles], F32))
        valD = es.enter_context(nc.sbuf_tensor([P, n_tiles], F32))
        out_sb = es.enter_context(nc.sbuf_tensor([P, n_tiles], F32))
        qS0 = es.enter_context(nc.psum_tensor([P, CW], F32))
        qS1 = es.enter_context(nc.psum_tensor([P, CW], F32))
        qD0 = es.enter_context(nc.psum_tensor([P, CW], F32))
        qD1 = es.enter_context(nc.psum_tensor([P, CW], F32))
        psA0 = es.enter_context(nc.psum_tensor([P, TPB * CB], F32))
        psA1 = es.enter_context(nc.psum_tensor([P, TPB * CB], F32))
        psB0 = es.enter_context(nc.psum_tensor([P, TPB * CB], F32))
        psB1 = es.enter_context(nc.psum_tensor([P, TPB * CB], F32))
        ph1_loadA = es.enter_context(nc.semaphore())
        ph1_loadB = es.enter_context(nc.semaphore())
        ph1_free = es.enter_context(nc.semaphore())
        pre_load = es.enter_context(nc.semaphore())
        ploadA = es.enter_context(nc.semaphore())
        ploadB = es.enter_context(nc.semaphore())
        mloadA = es.enter_context(nc.semaphore())
        mloadB = es.enter_context(nc.semaphore())
        ydone = es.enter_context(nc.semaphore())
        ps_done = es.enter_context(nc.semaphore())
        pd_done = es.enter_context(nc.semaphore())
        r_sv = es.enter_context(nc.semaphore())  # src relus on DVE
        r_sa = es.enter_context(nc.semaphore())  # src relus on ACT
        r_dv = es.enter_context(nc.semaphore())  # dst relus on DVE
        r_da = es.enter_context(nc.semaphore())  # dst relus on ACT
        seldone = es.enter_context(nc.semaphore())
        dvedone = es.enter_context(nc.semaphore())
        vchain = es.enter_context(nc.semaphore())
        fin = es.enter_context(nc.semaphore())
        block = es.enter_context(nc.Block())

        emb_bufs = [emb0, emb1]
        pS_bufs = [pS0, pS1]
        pD_bufs = [pD0, pD1]
        ohpS_bufs = [ohpS0, ohpS1]
        ohpD_bufs = [ohpD0, ohpD1]
        mb_bufs = [mb0, mb1]
        qS = [qS0, qS1]
        qD = [qD0, qD1]
        psA = [psA0, psA1]
        psB = [psB0, psB1]
        ph1_load = [ph1_loadA, ph1_loadB]
        pload = [ploadA, ploadB]
        mload = [mloadA, mloadB]
        NPRE = 1
        # relu ownership: (side, chunk) -> engine 'v' (DVE) or 'a' (ACT)
        def r_owner(side, c):
            return "v" if (side, c) in DVE_RELU else "a"
        R_SEMS = {(0, "v"): r_sv, (0, "a"): r_sa, (1, "v"): r_dv, (1, "a"): r_da}
        R_PERB = {k: sum(1 for c in range(NCH) if r_owner(k[0], c) == k[1])
                  for k in R_SEMS}
        def r_cum(side, i, c):
            # cumulative count on (side, owner(side, c)) up to and incl (i, c)
            eng = r_owner(side, c)
            n = R_PERB[(side, eng)] * i
            n += sum(1 for cc in range(c + 1) if r_owner(side, cc) == eng)
            return R_SEMS[(side, eng)], n
        def wait_relus_done(eng_obj, side, i):
            # all of batch i's relus for `side` complete
            for e in ("v", "a"):
                pb = R_PERB[(side, e)]
                if pb:
                    eng_obj.wait_ge(R_SEMS[(side, e)], pb * (i + 1))

        @block.sync
        def _(sync):
            sync.dma_start(out=lhsT3[:], in_=lhsT3_in[:]).then_inc(pre_load, 16)
            for k in range(n_emb):
                if k >= 2:
                    sync.wait_ge(ph1_free, k - 1)
                sync.dma_start(
                    out=emb_bufs[k % 2][:],
                    in_=embeds[k * 2048:(k + 1) * 2048, :].rearrange(
                        "(j p) d -> p j d", p=P
                    ),
                ).then_inc(ph1_load[k % 2], 16)
            for i in range(nbatch):
                if i >= 2:
                    sync.wait_ge(ps_done, NCH * (i - 1))
                    sync.wait_ge(pd_done, NCH * (i - 1))

                sync.dma_start(out=pS_bufs[i % 2][:], in_=p3_both[i, 0:3]).then_inc(pload[i % 2], 16)
                sync.dma_start(out=pD_bufs[i % 2][:], in_=p3_both[i, 3:6]).then_inc(pload[i % 2], 16)
                if i >= 2:
                    sync.wait_ge(dvedone, i - 1)
                sync.dma_start(out=mb_bufs[i % 2][:], in_=mask_in[i]).then_inc(mload[i % 2], 16)
            sync.wait_ge(fin, 1)
            sync.dma_start(out=y[:], in_=out_sb[:]).then_inc(ydone, 16)

        def relu_dve(vector, i, side, c):
            q = (qS if side == 0 else qD)[(i * NCH + c) % 2]
            ohp = (ohpS_bufs if side == 0 else ohpD_bufs)[i % 2]
            sem, val = r_cum(side, i, c)
            vector.tensor_scalar(
                out=ohp[:, c * CW:(c + 1) * CW],
                in0=q[:],
                scalar1=1.0,
                scalar2=0.0,
                op0=ALU.add,
                op1=ALU.max,
            ).then_inc(sem, 1)

        def relu_act(scalar, i, side, c):
            q = (qS if side == 0 else qD)[(i * NCH + c) % 2]
            ohp = (ohpS_bufs if side == 0 else ohpD_bufs)[i % 2]
            sem, val = r_cum(side, i, c)
            scalar.activation(
                out=ohp[:, c * CW:(c + 1) * CW],
                in_=q[:],
                func=ACTF.Relu,
                bias=1.0,
                scale=1.0,
            ).then_inc(sem, 1)

        @block.scalar
        def _(scalar):
            for i in range(nbatch):
                if i >= 2:
                    scalar.wait_ge(seldone, i - 1)  # ohp bufs free
                for c in range(NCH):
                    scalar.wait_ge(pd_done, i * NCH + c + 1)
                    relu_act(scalar, i, 1, c)
                for c in range(NCH):
                    if (0, c) not in DVE_RELU:
                        scalar.wait_ge(ps_done, i * NCH + c + 1)
                        relu_act(scalar, i, 0, c)

        @block.vector
        def _(vector):
            # ---- phase 1 ----
            vector.memset(S[:, 784:NBLK * CB], 0.0)
            for k in range(n_emb):
                vector.wait_ge(ph1_load[k % 2], 16 * (k // 2 + 1))
                vector.tensor_reduce(
                    out=S[:, k * 16:(k + 1) * 16],
                    in_=emb_bufs[k % 2][:].rearrange("p (j d) -> p j d", d=D),
                    op=ALU.add,
                    axis=mybir.AxisListType.X,
                ).then_inc(ph1_free, 1)

            # ---- phase 2 ----
            def relus(i):
                if i >= 2:
                    vector.wait_ge(seldone, i - 1)
                for c in range(NCH):
                    if (0, c) in DVE_RELU:
                        vector.wait_ge(ps_done, i * NCH + c + 1)
                        relu_dve(vector, i, 0, c)

            vch = [0]

            def select(i):
                vector.wait_ge(seldone, i + 1)
                vector.wait_ge(mload[i % 2], 16 * (i // 2 + 1))
                if i >= 1:
                    vector.wait_ge(vchain, vch[0])  # dS WAR vs reduce_s(i-1)
                vector.tensor_tensor(
                    out=dS[:], in0=psA[i % 2][:], in1=mb_bufs[i % 2][:, :TPB * CB],
                    op=ALU.mult,
                ).then_inc(vchain, 1)
                vch[0] += 1
                vector.wait_ge(vchain, vch[0])      # dS RAW
                vector.tensor_reduce(
                    out=valS[:, i * TPB:(i + 1) * TPB],
                    in_=dS[:].rearrange("p (t c) -> p t c", c=CB),
                    op=ALU.add,
                    axis=mybir.AxisListType.X,
                ).then_inc(vchain, 1)
                vch[0] += 1
                if i >= 1:
                    vector.wait_ge(dvedone, i)      # dD WAR vs reduce_d(i-1)
                vector.tensor_tensor(
                    out=dD[:], in0=psB[i % 2][:], in1=mb_bufs[i % 2][:, TPB * CB:],
                    op=ALU.mult,
                ).then_inc(vchain, 1)
                vch[0] += 1
                vector.wait_ge(vchain, vch[0])      # dD RAW
                vector.tensor_reduce(
                    out=valD[:, i * TPB:(i + 1) * TPB],
                    in_=dD[:].rearrange("p (t c) -> p t c", c=CB),
                    op=ALU.add,
                    axis=mybir.AxisListType.X,
                ).then_inc(dvedone, 1)

            for i in range(nbatch):
                relus(i)
                if i >= 1:
                    select(i - 1)
            select(nbatch - 1)
            vector.wait_ge(vchain, vch[0])
            vector.wait_ge(dvedone, nbatch)
            vector.tensor_tensor(
                out=out_sb[:], in0=valS[:], in1=valD[:], op=ALU.subtract,
            ).then_inc(vchain, 1)
            vch[0] += 1
            vector.wait_ge(vchain, vch[0])
            vector.tensor_scalar(
                out=out_sb[:], in0=out_sb[:], scalar1=INV_SQ, scalar2=None,
                op0=ALU.mult,
            ).then_inc(fin, 1)

        @block.tensor
        def _(tensor):
            tensor.wait_ge(ph1_free, n_emb)
            tensor.wait_ge(pre_load, 16 * NPRE)
            for i in range(nbatch):
                tensor.wait_ge(pload[i % 2], 32 * (i // 2 + 1))
                for c in range(NCH):
                    q = i * NCH + c
                    if q >= 2:
                        i2, c2 = divmod(q - 2, NCH)
                        sem2, n2 = r_cum(0, i2, c2)
                        tensor.wait_ge(sem2, n2)  # qS bank free
                    tensor.matmul(
                        out=qS[q % 2][:],
                        lhsT=lhsT3[:],
                        rhs=pS_bufs[i % 2][:, c * CW:(c + 1) * CW],
                        start=True, stop=True,
                    ).then_inc(ps_done, 1)
                for c in range(NCH):
                    q = i * NCH + c
                    if q >= 2:
                        i2, c2 = divmod(q - 2, NCH)
                        sem2, n2 = r_cum(1, i2, c2)
                        tensor.wait_ge(sem2, n2)
                    tensor.matmul(
                        out=qD[q % 2][:],
                        lhsT=lhsT3[:],
                        rhs=pD_bufs[i % 2][:, c * CW:(c + 1) * CW],
                        start=True, stop=True,
                    ).then_inc(pd_done, 1)
                wait_relus_done(tensor, 0, i)
                wait_relus_done(tensor, 1, i)
                if i >= 2:
                    tensor.wait_ge(dvedone, i - 1)
                for j in range(TPB):
                    bs, bd = sched[i * TPB + j]
                    tensor.matmul(
                        out=psA[i % 2][:, j * CB:(j + 1) * CB],
                        lhsT=ohpS_bufs[i % 2][:, j * P:(j + 1) * P],
                        rhs=S[:, bs * CB:(bs + 1) * CB],
                        start=True, stop=True,
                    )
                    mm = tensor.matmul(
                        out=psB[i % 2][:, j * CB:(j + 1) * CB],
                        lhsT=ohpD_bufs[i % 2][:, j * P:(j + 1) * P],
                        rhs=S[:, bd * CB:(bd + 1) * CB],
                        start=True, stop=True,
                    )
                    if j == TPB - 1:
                        mm.then_inc(seldone, 1)

    return nc


def _prep(src_flat, dst_flat):
    E = src_flat.shape[0]
    assert E % NCORES == 0
    Ec = E // NCORES
    NG = NBLK * NBLK

    cores = []
    counts = np.zeros((NCORES, NG), np.int64)
    for i in range(NCORES):
        s = src_flat[i * Ec:(i + 1) * Ec].astype(np.int64)
        d = dst_flat[i * Ec:(i + 1) * Ec].astype(np.int64)
        g = (s >> 12) * NBLK + (d >> 12)
        order = np.argsort(g, kind="stable")
        cores.append((s[order], d[order], g[order], order + i * Ec))
        counts[i] = np.bincount(g, minlength=NG)

    gmax = counts.max(axis=0)
    tiles_per_group = (gmax + P - 1) // P
    n_tiles = int(tiles_per_group.sum())
    n_tiles_p = ((n_tiles + TPB - 1) // TPB) * TPB

    sched = []
    for gi in range(NG):
        sched.extend([(gi // NBLK, gi % NBLK)] * int(tiles_per_group[gi]))
    sched.extend([(0, 0)] * (n_tiles_p - n_tiles))

    slot_base = np.zeros(NG, np.int64)
    np.cumsum(tiles_per_group[:-1] * P, out=slot_base[1:])
    n_slots = n_tiles_p * P
    nbatch = n_tiles_p // TPB

    per_core = []
    for i in range(NCORES):
        s, d, g, orig = cores[i]
        cstart = np.zeros(NG, np.int64)
        np.cumsum(counts[i][:-1], out=cstart[1:])
        within = np.arange(Ec) - cstart[g]
        slot = slot_base[g] + within
        src_s = np.zeros(n_slots, np.int64)
        dst_s = np.zeros(n_slots, np.int64)
        src_s[slot] = s
        dst_s[slot] = d

        def p3(arr):
            pe = (arr & 127).astype(np.float32).reshape(nbatch, TPB * P)
            out = np.empty((nbatch, 3, TPB * P), np.float32)
            out[:, 0, :] = pe
            out[:, 1, :] = -(pe * pe)
            out[:, 2, :] = 1.0
            return out

        def cmask(arr):
            # [nbatch, P(edge-in-tile), TPB*CB]: one-hot of c_e along CB
            c = ((arr >> 7) & 31).astype(np.int8).reshape(nbatch, TPB, P)
            oh = (c[:, :, :, None] == np.arange(CB, dtype=np.int8)).astype(
                np.float32
            )  # [nbatch, TPB, P(e), CB]
            return np.ascontiguousarray(
                oh.transpose(0, 2, 1, 3).reshape(nbatch, P, TPB * CB)
            )

        per_core.append(
            dict(
                p3_both=np.concatenate([p3(src_s), p3(dst_s)], axis=1),
                mask_both=np.concatenate([cmask(src_s), cmask(dst_s)], axis=2),
                slot=slot,
                orig=orig,
            )
        )
    return per_core, sched, n_tiles_p


def kernel(node_embeds, src_idx, dst_idx):
    node_embeds = np.asarray(node_embeds, dtype=np.float32)
    src_idx = np.asarray(src_idx)
    dst_idx = np.asarray(dst_idx)
    T, E = src_idx.shape
    n_nodes = node_embeds.shape[0]

    src_flat = src_idx.reshape(-1).astype(np.int64)
    dst_flat = dst_idx.reshape(-1).astype(np.int64)
    per_core, sched, n_tiles_p = _prep(src_flat, dst_flat)

    emb_pad = np.zeros((VPAD, D), np.float32)
    emb_pad[:n_nodes] = node_embeds

    iota = np.arange(P, dtype=np.float32)
    lhsT3 = np.stack([2.0 * iota, np.ones(P, np.float32), -(iota * iota)])

    nc = _build_nc(n_tiles_p, sched)
    in_maps = []
    for i in range(NCORES):
        pc = per_core[i]
        in_maps.append(
            {
                "embeds": emb_pad,
                "p3_both": pc["p3_both"],
                "mask_in": pc["mask_both"],
                "lhsT3": lhsT3,
            }
        )
    res = run_bass_kernel_spmd(nc, in_maps, list(range(NCORES)))

    out_flat = np.zeros(T * E, np.float32)
    for i in range(NCORES):
        pc = per_core[i]
        yv = res.results[i]["y"]
        slot_vals = np.ascontiguousarray(yv.T).reshape(-1)
        out_flat[pc["orig"]] = slot_vals[pc["slot"]]
    return out_flat.reshape(T, E)
